# revision 1
# baseline (speedup 1.0000x reference)
"""Trainium2 Bass kernel for one transformer block (nn_Block_25838523252853).

Full inputs in, full output out. Sharding: the 4096 tokens (B=4 x L=1024)
are split 8 ways -- each core owns 512 tokens (half of one sequence).
Attention needs full-sequence K/V, so each core computes skip-linear/LN1/K/V
for its WHOLE sequence (2x duplicated work within a pair) and q/proj/MLP for
its own half only. No collectives; all 8 cores run one SPMD NEFF.

Device layout: activations are channel-major bf16 ([C_part, T_free] tiles),
weights in natural [inC, outC] layout as matmul lhsT. LayerNorm reductions
(over channels = partitions) use ones-vector matmuls; per-token scalars are
broadcast across partitions with tiny K=1 matmuls. Softmax skips the max
subtraction (scores are bounded ~|9| for this problem) and gets the row sums
for free from a ones-column appended to V.
"""

import numpy as np
import ml_dtypes

import concourse.bass as bass
import concourse.tile as tile
from concourse import bacc, mybir
from concourse.bass_utils import run_bass_kernel_spmd

F32 = mybir.dt.float32
BF16 = mybir.dt.bfloat16
FP16 = mybir.dt.float16

DIM = 1024
HEADS = 16
HD = 64
HIDDEN = 4096
EPS = 1e-5
SCALE = HD ** -0.5
B, L = 4, 1024
T = 512          # tokens owned per core
P = 128
NC = 8

_BUILT = None


def _emit_ln(nc, tc, ppool, tpool, raw, sq, gcol, bcol, out_tiles, out_dtype, n_feat):
    """LayerNorm over channels (partition axis) in channel-major layout.

    raw: list of 8 [128, T] bf16 tiles (the pre-norm activations)
    sq:  list of 8 [128, T] fp16 tiles (elementwise squares of raw)
    gcol/bcol: [128, 1] f32 APs (per-channel gamma/beta, per partition)
                given per m-tile via gcol(m), bcol(m)
    out_tiles(m) -> destination [128, T] tile of out_dtype
    """
    ones_b = _emit_ln.ones_b          # [128,1] bf16
    ones_h = _emit_ln.ones_h          # [128,1] fp16
    ones_row = _emit_ln.ones_row      # [1,128] f32
    nk = len(raw)
    inv_n = 1.0 / n_feat
    stats = ppool.tile([P, T], F32, tag="st", name="st", bufs=1)
    for k in range(nk):
        nc.tensor.matmul(stats[0:1, :], lhsT=ones_b, rhs=raw[k],
                         start=(k == 0), stop=(k == nk - 1))
    for k in range(nk):
        nc.tensor.matmul(stats[32:33, :], lhsT=ones_h, rhs=sq[k],
                         start=(k == 0), stop=(k == nk - 1))
    # msq = (sum/n)^2 and s2n = sumsq/n straight off PSUM
    msq = tpool.tile([1, T], F32, tag="ln_msq", name="ln_msq", bufs=1)
    nc.scalar.activation(msq, stats[0:1, :], mybir.ActivationFunctionType.Square,
                         scale=inv_n)
    s2n = tpool.tile([1, T], F32, tag="ln_s2", name="ln_s2", bufs=1)
    nc.scalar.mul(s2n, stats[32:33, :], inv_n)
    var = tpool.tile([1, T], F32, tag="ln_var", name="ln_var", bufs=1)
    nc.vector.tensor_tensor(var, s2n, msq, mybir.AluOpType.subtract)
    lnv = tpool.tile([1, T], F32, tag="ln_std", name="ln_std", bufs=1)
    nc.scalar.activation(lnv, var, mybir.ActivationFunctionType.Ln,
                         bias=_emit_ln.eps_t)
    rstd = tpool.tile([1, T], F32, tag="ln_rstd", name="ln_rstd", bufs=1)
    nc.scalar.activation(rstd, lnv, mybir.ActivationFunctionType.Exp, scale=-0.5)
    # B = -(sum/n)*rstd broadcast: fold -1/n into the broadcast lhsT constant
    mr = tpool.tile([1, T], F32, tag="ln_negmr", name="ln_negmr", bufs=1)
    nc.vector.tensor_tensor(mr, stats[0:1, :], rstd, mybir.AluOpType.mult)
    a_bc = ppool.tile([P, T], F32, tag="mm", name="mm")
    nc.tensor.matmul(a_bc, lhsT=ones_row, rhs=rstd, start=True, stop=True)
    b_bc = ppool.tile([P, T], F32, tag="mm", name="mm")
    nc.tensor.matmul(b_bc, lhsT=_emit_ln.neginv_row[:, :, n_feat], rhs=mr,
                     start=True, stop=True)
    a_sb = tpool.tile([P, T], BF16, tag="ln_asb", name="ln_asb", bufs=1)
    nc.vector.tensor_copy(out=a_sb, in_=a_bc)
    b_sb = tpool.tile([P, T], BF16, tag="ln_bsb", name="ln_bsb", bufs=1)
    nc.vector.tensor_copy(out=b_sb, in_=b_bc)
    for m in range(nk):
        t1 = tpool.tile([P, T], BF16, tag="ln_t1", name="ln_t1", bufs=2)
        nc.vector.tensor_tensor(t1, raw[m], a_sb, mybir.AluOpType.mult)
        nc.vector.tensor_tensor(t1, t1, b_sb, mybir.AluOpType.add)
        nc.scalar.activation(out_tiles(m), t1, mybir.ActivationFunctionType.Identity,
                             bias=bcol(m), scale=gcol(m))


def _heat(nc, ppool, src_a, src_b, n):
    """Dead matmuls that keep the PE HAM-warm across a known stall."""
    hp = ppool.tile([P, T], F32, tag="st", name="heat", bufs=1)
    for i in range(n):
        nc.tensor.matmul(hp, lhsT=src_a[:, 0:P], rhs=src_b,
                         start=(i == 0), stop=(i == n - 1))


def build():
    """Build + bacc-compile the SPMD program. Cached per process."""
    global _BUILT
    if _BUILT is not None:
        return _BUILT

    nc = bacc.Bacc("TRN2", target_bir_lowering=False, debug=False, num_devices=NC)

    d_xs = nc.dram_tensor("xs", [2 * DIM, T], BF16, kind="ExternalInput").ap()
    ccK_in = [nc.dram_tensor(f"ccK_in{i}", [DIM // 2, T], BF16).ap() for i in range(2)]
    ccK_out = [nc.dram_tensor(f"ccK_out{i}", [DIM, T], BF16).ap() for i in range(2)]
    ccV_in = nc.dram_tensor("ccV_in", [DIM, T], BF16).ap()
    ccV_out = nc.dram_tensor("ccV_out", [2 * DIM, T], BF16).ap()
    d_wsk = nc.dram_tensor("wsk", [2 * DIM, DIM], BF16, kind="ExternalInput").ap()
    d_wq = nc.dram_tensor("wq", [DIM, DIM], BF16, kind="ExternalInput").ap()
    d_wk = nc.dram_tensor("wk", [DIM, DIM], BF16, kind="ExternalInput").ap()
    d_wv = nc.dram_tensor("wv", [DIM, DIM], BF16, kind="ExternalInput").ap()
    d_wp = nc.dram_tensor("wp", [DIM, DIM], BF16, kind="ExternalInput").ap()
    d_w1 = nc.dram_tensor("w1", [DIM, HIDDEN], BF16, kind="ExternalInput").ap()
    d_w2 = nc.dram_tensor("w2", [HIDDEN, DIM], BF16, kind="ExternalInput").ap()
    d_lnp = nc.dram_tensor("lnp", [P, 104], F32, kind="ExternalInput").ap()
    d_sel16 = nc.dram_tensor("sel16", [HEADS, HEADS * HD], BF16, kind="ExternalInput").ap()
    d_out = nc.dram_tensor("out", [DIM, T], F32, kind="ExternalOutput").ap()

    # lnp column layout (each group of 8/32 cols is one [1024]/[4096] vector,
    # channel c -> [c % 128, base + c // 128])
    C_LN1G, C_LN1B, C_LN2G, C_LN2B, C_LN3G, C_LN3B = 0, 8, 16, 24, 32, 40
    C_SKB, C_PRB, C_F2B, C_F1B = 48, 56, 64, 72

    EXPW = 2 * T  # score/exp tiles span two k-tiles

    with tile.TileContext(nc, pool_alloc_mode="queue") as tc:
        with tc.tile_pool(name="glob", bufs=1) as gpool, \
             tc.tile_pool(name="tmp", bufs=2) as tpool, \
             tc.tile_pool(name="ps", bufs=3, space="PSUM") as ppool:

            lnp = gpool.tile([P, 104], F32, tag="lnp", name="lnp")
            ones_b = gpool.tile([P, 1], BF16, tag="ones_b", name="ones_b")
            nc.vector.memset(ones_b, 1.0)
            ones_h = gpool.tile([P, 1], FP16, tag="ones_h", name="ones_h")
            nc.vector.memset(ones_h, 1.0)
            ones_row = gpool.tile([1, P], F32, tag="ones_row", name="ones_row")
            nc.vector.memset(ones_row, 1.0)
            sel16 = gpool.tile([HEADS, HEADS * HD], BF16, tag="sel16", name="sel16")
            eps_t = gpool.tile([1, 1], F32, tag="eps_t", name="eps_t")
            nc.vector.memset(eps_t, EPS)
            neginv = gpool.tile([1, P], F32, tag="neginv", name="neginv")
            nc.vector.memset(neginv, -1.0 / DIM)

            class _NegRow:
                def __getitem__(self, key):
                    return neginv
            _emit_ln.neginv_row = _NegRow()
            _emit_ln.eps_t = eps_t
            _emit_ln.nc_ref = nc
            _emit_ln.ones_b = ones_b
            _emit_ln.ones_h = ones_h
            _emit_ln.ones_row = ones_row

            x2n = [gpool.tile([P, T], BF16, tag=f"x2n{m}", name=f"x2n{m}") for m in range(8)]
            xo = [gpool.tile([P, T], F32, tag=f"xo{m}", name=f"xo{m}") for m in range(8)]

            with tc.tile_pool(name="span1", bufs=1) as spool:
                x1n = [spool.tile([P, T], BF16, tag=f"x1n_{k}", name=f"x1n_{k}")
                       for k in range(8)]
                oT = [spool.tile([P, T], BF16, tag=f"oT{m}", name=f"oT{m}") for m in range(8)]

                # qkv weights: prefetch during phase A (their pool outlives pha)
                wqkvp = tc.alloc_tile_pool(name="wqkv", bufs=1)
                if True:
                    wq = [wqkvp.tile([P, DIM], BF16, tag=f"wq{k}", name=f"wq{k}") for k in range(8)]
                    wk = [wqkvp.tile([P, DIM], BF16, tag=f"wk{k}", name=f"wk{k}") for k in range(8)]

                    # ---- Phase A: skip-concat linear + LN1 (own half only) ----
                    with tc.tile_pool(name="pha", bufs=1) as apool:
                        wsk = [apool.tile([P, DIM], BF16, tag=f"wsk{k}", name=f"wsk{k}")
                               for k in range(16)]
                        xs = [apool.tile([P, T], BF16, tag=f"xsh{k}", name=f"xsh{k}")
                              for k in range(16)]
                        for k in range(16):
                            # interleave so matmul k can start as soon as its pair lands
                            nc.sync.dma_start(out=wsk[k], in_=d_wsk[k * P:(k + 1) * P, :])
                            nc.sync.dma_start(out=xs[k], in_=d_xs[k * P:(k + 1) * P, :])
                            if k == 0:
                                nc.gpsimd.dma_start(out=lnp, in_=d_lnp)
                                nc.gpsimd.dma_start(out=sel16, in_=d_sel16)
                        # prefetch q/k weights behind phase-A tiles (v comes in phase B)
                        for k in range(8):
                            nc.sync.dma_start(out=wq[k], in_=d_wq[k * P:(k + 1) * P, :])
                            nc.sync.dma_start(out=wk[k], in_=d_wk[k * P:(k + 1) * P, :])
                        raw = [apool.tile([P, T], BF16, tag=f"raw{m}", name=f"raw{m}")
                               for m in range(8)]
                        sq = [apool.tile([P, T], FP16, tag=f"sq{m}", name=f"sq{m}")
                              for m in range(8)]
                        for m in range(8):
                            ps = ppool.tile([P, T], F32, tag="mm", name="mm")
                            for k in range(16):
                                nc.tensor.matmul(
                                    ps, lhsT=wsk[k][:, m * P:(m + 1) * P], rhs=xs[k],
                                    start=(k == 0), stop=(k == 15))
                            nc.scalar.activation(
                                raw[m], ps, mybir.ActivationFunctionType.Identity,
                                bias=lnp[:, C_SKB + m:C_SKB + m + 1])
                            nc.scalar.activation(
                                sq[m], raw[m], mybir.ActivationFunctionType.Square)
                        _emit_ln(nc, tc, ppool, tpool, raw, sq,
                                 lambda m: lnp[:, C_LN1G + m:C_LN1G + m + 1],
                                 lambda m: lnp[:, C_LN1B + m:C_LN1B + m + 1],
                                 lambda m: x1n[m], BF16, DIM)

                    # ---- Phase B: local k/v, AllGather within the pair, qkv ----
                    with tc.tile_pool(name="phb", bufs=1) as bpool, \
                         tc.tile_pool(name="atmp", bufs=2) as atpool, \
                         tc.tile_pool(name="exps", bufs=7) as xpool:
                        wv = [wqkvp.tile([P, DIM], BF16, tag=f"wv{k}", name=f"wv{k}")
                              for k in range(8)]
                        for k in range(8):
                            nc.sync.dma_start(out=wv[k], in_=d_wv[k * P:(k + 1) * P, :])

                        # local K (channel-major) -> ccK_in, AG issued ASAP
                        kloc = [bpool.tile([P, T], BF16, tag=f"kl{m}", name=f"kl{m}")
                                for m in range(8)]
                        for half in range(2):
                            for mi in range(4):
                                m = half * 4 + mi
                                pk = ppool.tile([P, T], F32, tag="mm", name="mm")
                                for k in range(8):
                                    nc.tensor.matmul(pk, lhsT=wk[k][:, m * P:(m + 1) * P],
                                                     rhs=x1n[k], start=(k == 0), stop=(k == 7))
                                nc.vector.tensor_copy(out=kloc[m], in_=pk)
                                nc.sync.dma_start(out=ccK_in[half][mi * P:(mi + 1) * P, :],
                                                  in_=kloc[m])
                            if half == 0:
                                nc.gpsimd.collective_compute(
                                    "AllGather", mybir.AluOpType.bypass,
                                    replica_groups=[[0, 1], [2, 3], [4, 5], [6, 7]],
                                    ins=[ccK_in[0][:]], outs=[ccK_out[0][:]],
                                )
                        # local V (token-major) -> ccV_in (overlaps K collective)
                        vv_in = ccV_in.rearrange("(t two) c -> t (two c)", two=2)
                        vloc = [bpool.tile([P, 2 * T], BF16, tag=f"vl{kt}", name=f"vl{kt}")
                                for kt in range(4)]
                        for kt in range(4):
                            for half in range(2):
                                ps = ppool.tile([P, T], F32, tag="mm", name="mm")
                                for k in range(8):
                                    nc.tensor.matmul(
                                        ps, lhsT=x1n[k][:, kt * P:(kt + 1) * P],
                                        rhs=wv[k][:, half * T:(half + 1) * T],
                                        start=(k == 0), stop=(k == 7))
                                nc.vector.tensor_copy(
                                    out=vloc[kt][:, half * T:(half + 1) * T], in_=ps)
                            nc.sync.dma_start(out=vv_in[kt * P:(kt + 1) * P, :],
                                              in_=vloc[kt])
                        nc.gpsimd.collective_compute(
                            "AllGather", mybir.AluOpType.bypass,
                            replica_groups=[[0, 1], [2, 3], [4, 5], [6, 7]],
                            ins=[ccV_in[:]], outs=[ccV_out[:]],
                        )
                        nc.gpsimd.collective_compute(
                            "AllGather", mybir.AluOpType.bypass,
                            replica_groups=[[0, 1], [2, 3], [4, 5], [6, 7]],
                            ins=[ccK_in[1][:]], outs=[ccK_out[1][:]],
                        )
                        # q for own tokens (overlaps the collectives)
                        qT = [bpool.tile([P, T], BF16, tag=f"qT{m}", name=f"qT{m}")
                              for m in range(8)]
                        for m in range(8):
                            ps = ppool.tile([P, T], F32, tag="mm", name="mm")
                            for k in range(8):
                                nc.tensor.matmul(ps, lhsT=wq[k][:, m * P:(m + 1) * P],
                                                 rhs=x1n[k], start=(k == 0), stop=(k == 7))
                            nc.vector.tensor_copy(out=qT[m], in_=ps)

                        # reload gathered K/V (uniform across cores; k-token order
                        # is attention-invariant)
                        kT = [[bpool.tile([P, T], BF16, tag=f"kT_{m}_{b}", name=f"kT_{m}_{b}")
                               for b in range(2)] for m in range(8)]
                        for half in range(2):
                            for b in range(2):
                                for mi in range(4):
                                    m = half * 4 + mi
                                    nc.sync.dma_start(
                                        out=kT[m][b],
                                        in_=ccK_out[half][b * (DIM // 2) + mi * P:
                                                          b * (DIM // 2) + (mi + 1) * P, :])
                        v_sb = [bpool.tile([P, HEADS * (HD + 1)], BF16, tag=f"v{kt}",
                                           name=f"v{kt}")
                                for kt in range(8)]
                        for kt in range(8):
                            b, ktl = kt // 4, kt % 4
                            vv_out = ccV_out[b * DIM:(b + 1) * DIM, :].rearrange(
                                "(t two) c -> t (two c)", two=2)
                            v3 = v_sb[kt].rearrange("p (h c) -> p h c", c=HD + 1)
                            nc.sync.dma_start(
                                out=v3[:, :, 0:HD],
                                in_=vv_out[ktl * P:(ktl + 1) * P, :].rearrange(
                                    "p (h c) -> p h c", c=HD))
                            nc.vector.memset(v3[:, :, HD:HD + 1], 1.0)

                        # ---- Phase C: attention ----
                        # stage 1: per-head unnormalized o + sums (ACT does only Exp)
                        sums8 = [wqkvp.tile([8, T], F32, tag=f"wv{4 + g}", name=f"sums8_{g}")
                                 for g in range(2)]
                        rp8 = [None, None]
                        oUs = []
                        for hd in range(HEADS):
                            m2, off = hd // 2, (hd % 2) * HD
                            exp_tiles = []
                            for kp in range(4):
                                ps2 = ppool.tile([P, EXPW], F32, tag="mm2", bufs=2, name="mm2")
                                for j in range(2):
                                    kt = 2 * kp + j
                                    th, col = kt // 4, (kt % 4) * P
                                    nc.tensor.matmul(
                                        ps2[:, j * T:(j + 1) * T],
                                        lhsT=kT[m2][th][off:off + HD, col:col + P],
                                        rhs=qT[m2][off:off + HD, :], start=True, stop=True)
                                e = xpool.tile([P, EXPW], BF16, tag="exp", name="exp")
                                nc.scalar.activation(e, ps2, mybir.ActivationFunctionType.Exp)
                                exp_tiles.append(e)
                            po = ppool.tile([P, T], F32, tag="mm", name="mm")
                            for kt in range(8):
                                nc.tensor.matmul(
                                    po[0:HD + 1, :],
                                    lhsT=v_sb[kt][:, hd * (HD + 1):(hd + 1) * (HD + 1)],
                                    rhs=exp_tiles[kt // 2][:, (kt % 2) * T:(kt % 2 + 1) * T],
                                    start=(kt == 0), stop=(kt == 7))
                            ou_tag = f"wq{hd}" if hd < 8 else f"wk{hd - 8}"
                            oU = wqkvp.tile([HD + 1, T], F32, tag=ou_tag, name=f"oU{hd}")
                            nc.vector.tensor_copy(out=oU, in_=po[0:HD + 1, :])
                            g, hg = hd // 8, hd % 8
                            nc.sync.dma_start(out=sums8[g][hg:hg + 1, :],
                                              in_=oU[HD:HD + 1, :])
                            oUs.append(oU)
                            if hd % 8 == 7:
                                # batched reciprocal for this group of 8 heads;
                                # heads of the other group keep the PE busy meanwhile
                                rpf = wqkvp.tile([8, T], F32, tag=f"wv{1 + g}",
                                                 name=f"rpf{g}")
                                nc.vector.reciprocal(rpf, sums8[g])
                                rp8[g] = wqkvp.tile([8, T], BF16, tag=f"wv{6 + g}",
                                                    name=f"rp8_{g}")
                                nc.vector.tensor_copy(out=rp8[g], in_=rpf)
                                for h2 in range(g * 8, g * 8 + 8):
                                    m2, off = h2 // 2, (h2 % 2) * HD
                                    bc = ppool.tile([P, T], F32, tag="mm", name="mm")
                                    nc.tensor.matmul(
                                        bc[0:HD, :],
                                        lhsT=sel16[0:8, h2 * HD:(h2 + 1) * HD],
                                        rhs=rp8[g], start=True, stop=True)
                                    nc.vector.tensor_tensor(oT[m2][off:off + HD, :],
                                                            oUs[h2][0:HD, :],
                                                            bc[0:HD, :],
                                                            mybir.AluOpType.mult)

                    # ---- Phase D: proj + residual + LN2 (+ w1 prefetch) ----
                    wqkvp.release()
                    w1pool = tc.alloc_tile_pool(name="w1p", bufs=1)
                    with tc.tile_pool(name="phd", bufs=1) as dpool:
                        wp = [dpool.tile([P, DIM], BF16, tag=f"wp{k}", name=f"wp{k}")
                              for k in range(8)]
                        for k in range(8):
                            nc.sync.dma_start(out=wp[k], in_=d_wp[k * P:(k + 1) * P, :])
                        w1 = [w1pool.tile([P, HIDDEN], BF16, tag=f"w1{k}", name=f"w1{k}")
                              for k in range(8)]
                        for k in range(8):
                            nc.sync.dma_start(out=w1[k], in_=d_w1[k * P:(k + 1) * P, :])
                        x2r = [dpool.tile([P, T], BF16, tag=f"x2r{m}", name=f"x2r{m}")
                               for m in range(8)]
                        x2sq = [dpool.tile([P, T], FP16, tag=f"x2sq{m}", name=f"x2sq{m}")
                                for m in range(8)]
                        for m in range(8):
                            ps = ppool.tile([P, T], F32, tag="mm", name="mm")
                            for k in range(8):
                                nc.tensor.matmul(ps, lhsT=wp[k][:, m * P:(m + 1) * P],
                                                 rhs=oT[k], start=(k == 0), stop=(k == 7))
                            t = tpool.tile([P, T], BF16, tag="pd", name="pd")
                            nc.scalar.activation(t, ps, mybir.ActivationFunctionType.Identity,
                                                 bias=lnp[:, C_PRB + m:C_PRB + m + 1])
                            nc.vector.tensor_tensor(x2r[m], t, x1n[m], mybir.AluOpType.add)
                            nc.scalar.activation(x2sq[m], x2r[m],
                                                 mybir.ActivationFunctionType.Square)
                        _emit_ln(nc, tc, ppool, tpool, x2r, x2sq,
                                 lambda m: lnp[:, C_LN2G + m:C_LN2G + m + 1],
                                 lambda m: lnp[:, C_LN2B + m:C_LN2B + m + 1],
                                 lambda m: x2n[m], BF16, DIM)

                # ---- Phase E: MLP + LN3 (hT reuses dead x1n/oT slots in span1) ----
                def _ht_tag(mm):
                    if mm < 8:
                        return f"x1n_{mm}"
                    if mm < 16:
                        return f"oT{mm - 8}"
                    return f"hTx{mm - 16}"
                hT = []
                for mm in range(32):
                    t_ = spool.tile([P, T], BF16, tag=_ht_tag(mm), name=f"hT{mm}")
                    hT.append(t_)
                for mm in range(32):
                    ps = ppool.tile([P, T], F32, tag="mm", name="mm")
                    for k in range(8):
                        nc.tensor.matmul(ps, lhsT=w1[k][:, mm * P:(mm + 1) * P],
                                         rhs=x2n[k], start=(k == 0), stop=(k == 7))
                    nc.scalar.activation(hT[mm], ps, mybir.ActivationFunctionType.Gelu,
                                         bias=lnp[:, C_F1B + mm:C_F1B + mm + 1])
                w1pool.release()

                phx = tc.alloc_tile_pool(name="phx", bufs=1)
                w2pool = tc.alloc_tile_pool(name="w2s", bufs=1)
                x3r = [phx.tile([P, T], BF16, tag=f"x3r{m}", name=f"x3r{m}") for m in range(8)]
                x3sq = [phx.tile([P, T], FP16, tag=f"x3sq{m}", name=f"x3sq{m}") for m in range(8)]
                w2res = [None] * 32
                for mh in range(2):
                    pss = [ppool.tile([P, EXPW], F32, tag="mm2", bufs=2, name="mm2")
                           for _ in range(2)]
                    for k in range(32):
                        if mh == 0:
                            w2t = w2pool.tile([P, DIM], BF16, tag=f"w2_{k}",
                                              name=f"w2_{k}")
                            nc.sync.dma_start(out=w2t, in_=d_w2[k * P:(k + 1) * P, :])
                            w2res[k] = w2t
                        else:
                            w2t = w2res[k]
                        for j in range(4):
                            m = mh * 4 + j
                            nc.tensor.matmul(pss[j // 2][:, (j % 2) * T:(j % 2 + 1) * T],
                                             lhsT=w2t[:, m * P:(m + 1) * P],
                                             rhs=hT[k], start=(k == 0), stop=(k == 31))
                    for j in range(4):
                        m = mh * 4 + j
                        t = tpool.tile([P, T], BF16, tag="pd", name="pd")
                        nc.scalar.activation(t, pss[j // 2][:, (j % 2) * T:(j % 2 + 1) * T],
                                             mybir.ActivationFunctionType.Identity,
                                             bias=lnp[:, C_F2B + m:C_F2B + m + 1])
                        nc.vector.tensor_tensor(x3r[m], t, x2n[m], mybir.AluOpType.add)
                        nc.scalar.activation(x3sq[m], x3r[m],
                                             mybir.ActivationFunctionType.Square)
                _emit_ln(nc, tc, ppool, tpool, x3r, x3sq,
                         lambda m: lnp[:, C_LN3G + m:C_LN3G + m + 1],
                         lambda m: lnp[:, C_LN3B + m:C_LN3B + m + 1],
                         lambda m: xo[m], F32, DIM)

                w2pool.release()
                phx.release()

            vout = d_out.rearrange("(t p) c -> t p c", p=P)
            for m in range(8):
                eng = nc.sync if m % 2 == 0 else nc.gpsimd
                eng.dma_start(out=vout[m], in_=xo[m])

    # Steer the act-table selector: keep dict ORDER (act_func_set_id is the
    # positional index into act_info.json) but hide Exp/Ln from the small
    # tables so both resolve to the combined natural_log_exp_and_others set
    # and the attention/LN loop stops thrashing table loads.
    import concourse.hw_specs as hw_specs
    tabs = dict(hw_specs.get_activation_tables("gen3"))
    EXP = mybir.ActivationFunctionType.Exp
    LN = mybir.ActivationFunctionType.Ln
    steered = {}
    for name, fns in tabs.items():
        fns = set(fns)
        if name != "natural_log_exp_and_others":
            fns.discard(EXP)
            fns.discard(LN)
        steered[name] = fns
    import functools
    _orig = hw_specs.get_activation_tables
    patched = functools.lru_cache(None)(
        lambda arch: steered if arch == "gen3" else _orig(arch))
    hw_specs.get_activation_tables = patched
    import concourse.bacc as bacc_mod
    bacc_mod.get_activation_tables = patched

    nc.compile()
    _BUILT = nc
    return nc


def _pack_col(vec, ncols):
    """[N] per-channel vector -> [128, N//128] tile layout (channel c -> [c%128, c//128])."""
    return np.ascontiguousarray(vec.reshape(ncols, P).T.astype(np.float32))


def _prep_in_maps(inputs):
    bf = ml_dtypes.bfloat16
    x = np.asarray(inputs["x"], np.float32)
    skip = np.asarray(inputs["skip"], np.float32)
    xs = np.concatenate([x, skip], axis=2)          # [4, 1024, 2048]

    wsk = np.asarray(inputs["skip_w"], np.float32).astype(bf)
    qkv = np.asarray(inputs["qkv_w"], np.float32)
    wq = (qkv[:, :DIM] * SCALE).astype(bf)
    wk = np.ascontiguousarray(qkv[:, DIM:2 * DIM]).astype(bf)
    wv = np.ascontiguousarray(qkv[:, 2 * DIM:]).astype(bf)
    wp = np.asarray(inputs["proj_w"], np.float32).astype(bf)
    w1 = np.asarray(inputs["fc1_w"], np.float32).astype(bf)
    w2 = np.asarray(inputs["fc2_w"], np.float32).astype(bf)

    lnp = np.zeros((P, 104), np.float32)
    lnp[:, 0:8] = _pack_col(np.asarray(inputs["ln1_g"], np.float32), 8)
    lnp[:, 8:16] = _pack_col(np.asarray(inputs["ln1_b"], np.float32), 8)
    lnp[:, 16:24] = _pack_col(np.asarray(inputs["ln2_g"], np.float32), 8)
    lnp[:, 24:32] = _pack_col(np.asarray(inputs["ln2_b"], np.float32), 8)
    lnp[:, 32:40] = _pack_col(np.asarray(inputs["ln3_g"], np.float32), 8)
    lnp[:, 40:48] = _pack_col(np.asarray(inputs["ln3_b"], np.float32), 8)
    lnp[:, 48:56] = _pack_col(np.asarray(inputs["skip_b"], np.float32), 8)
    lnp[:, 56:64] = _pack_col(np.asarray(inputs["proj_b"], np.float32), 8)
    lnp[:, 64:72] = _pack_col(np.asarray(inputs["fc2_b"], np.float32), 8)
    lnp[:, 72:104] = _pack_col(np.asarray(inputs["fc1_b"], np.float32), 32)

    sel16 = np.zeros((HEADS, HEADS * HD), np.float32)
    for h in range(HEADS):
        sel16[h % 8, h * HD:(h + 1) * HD] = 1.0

    in_maps = []
    for c in range(NC):
        b, h = c // 2, c % 2
        seq = xs[b][h * T:(h + 1) * T]               # own 512 tokens
        xsT = np.ascontiguousarray(seq.T).astype(bf)  # [2048, 512]
        in_maps.append({
            "xs": xsT, "wsk": wsk, "wq": wq, "wk": wk, "wv": wv,
            "wp": wp, "w1": w1, "w2": w2, "lnp": lnp, "sel16": sel16.astype(ml_dtypes.bfloat16),
        })
    return in_maps


def run(inputs, trace=False, trace_kwargs=None):
    nc = build()
    in_maps = _prep_in_maps(inputs)
    res = run_bass_kernel_spmd(nc, in_maps, core_ids=list(range(NC)),
                               trace=trace, **(trace_kwargs or {}))
    full = np.empty((B, L, DIM), np.float32)
    for c in range(NC):
        b, h = c // 2, c % 2
        full[b, h * T:(h + 1) * T, :] = res.results[c]["out"].T
    return full, res


def kernel(**inputs):
    out, _ = run(inputs, trace=False)
    return out

